# revision 39
# baseline (speedup 1.0000x reference)
"""Trainium2 Bass kernel for DTWFeatures.

Problem: x (64,3,1024), patts (32,3,32) -> out (64,32,1024)
  dist[b,p,l,t] = sqrt(max(|x[b,:,t]-patts[p,:,l]|^2, eps))
  DP:  D[l,t] = dist[l,t] + min(D[l-1,t], w*D[l,t-1], w*D[l-1,t-1])
  out[b,p,t] = D[L-1,t]

Strategy (8 cores, data-parallel over batch, 8 batches/core, 256 (b,p)
pairs/core = 2 groups of 128 partitions):
  * Rescale E[l,t] = D[l,t]*w^-(t-SHIFT): removes w from the recurrence.
  * Per DP row the recurrence is one DVE tensor_tensor_scan
    (op0=min, op1=add) with data0 = windowed min of the previous row.
  * Both 128-pair groups are processed by ONE 2049-wide scan per row: a
    boundary element with data1=1e30 blows the scan state up to ~1e30,
    which the next element's min() discards -- an exact chain reset.
  * E rows are stored bf16 (scan state stays fp32 internally), so the
    window-min TensorTensor runs in the DVE 2x_1p mode (2 elem/cycle).
    dist stays fp32.  Validated: L2 rel err ~3e-3 (tolerance 2e-2).
  * dist'^2 comes from a single K=33 fp32r matmul per 512-col chunk
    (fp32r = 1 cycle/row vs fp32's 4): rhs rows = x*-2w2inv (24),
    x2*w2inv (8), w2inv (1); lhsT = block-diag patts / batch indicators /
    p2+eps, one lhsT per group.  EPS=1.5e-2 floors d^2 against fp32r
    cancellation noise (HW sqrt(neg)=NaN).
  * ScalarE applies sqrt PSUM->SBUF; DVE is the bottleneck engine at
    ~105us busy (32 scans of 2049 + 30 bf16 window-mins).
"""

import os
import sys

if "/opt/trn_rl_repo" not in sys.path:
    sys.path.insert(0, "/opt/trn_rl_repo")
# the device path runs through jax's axon PJRT backend; make sure a
# harness-pinned JAX_PLATFORMS doesn't hide it (no-op if jax is already up)
if "jax" not in sys.modules and "axon" not in os.environ.get(
    "JAX_PLATFORMS", "axon"
):
    os.environ["JAX_PLATFORMS"] = "axon," + os.environ["JAX_PLATFORMS"]

import numpy as np

NB, ND, NP, NL, NT = 64, 3, 32, 32, 1024   # batch, xdim, n_patts, l_patts, T
NCORES = 8
BPC = NB // NCORES                     # 8 batches per core
RHO = 0.1
W = RHO ** (1.0 / NL)
SHIFT = 512.0
EPS = 1.5e-2                           # floors d^2 against fp32r noise
INF = 1.0e30
LARGE = 1.0e30                         # scan chain-reset boundary value
K = 33                                 # matmul contraction rows
NT2 = 2 * NT + 1                       # merged scan length (1024+1+1024)

_CACHE = {}


def _tables():
    if "tables" not in _CACHE:
        import ml_dtypes

        t = np.arange(NT, dtype=np.float64)
        w2inv = (W ** (-2.0 * (t - SHIFT))).astype(np.float32)
        wpos = (W ** (t - SHIFT)).astype(np.float32)
        W2INV24 = np.ascontiguousarray(np.tile(w2inv[None, :], (24, 1)) * -2.0)
        W2INVP8 = np.ascontiguousarray(np.tile(w2inv[None, :], (8, 1)))
        W2INV1 = np.ascontiguousarray(w2inv[None, :])
        WPOS2 = np.ascontiguousarray(np.tile(wpos[None, None, :], (128, 2, 1))).astype(ml_dtypes.bfloat16)
        EINF1 = np.full((128, 1), INF, np.float32).astype(ml_dtypes.bfloat16)
        _CACHE["tables"] = (W2INV24, W2INVP8, W2INV1, WPOS2, EINF1)
    return _CACHE["tables"]


def _lhbase(patts):
    """Full stationary lhsT per group: block-diagonal raw patts (rows 0:24,
    the -2 scale lives in the rhs), batch indicators (rows 24:32), and the
    p^2+eps row (row 32, paired with the rhs w2inv row)."""
    lhb = np.zeros((2, K, 128, NL), np.float32)
    pdl = np.transpose(patts, (1, 0, 2))  # (d, p, l)
    p2e = np.einsum("pdl,pdl->pl", patts, patts) + EPS
    for h in range(2):
        for bl in range(4):
            r = 12 * h + 3 * bl
            lhb[h, r : r + 3, 32 * bl : 32 * (bl + 1), :] = pdl
            lhb[h, 24 + 4 * h + bl, 32 * bl : 32 * (bl + 1), :] = 1.0
            lhb[h, 32, 32 * bl : 32 * (bl + 1), :] = p2e
    return lhb


def _build():
    if "nc" in _CACHE:
        return _CACHE["nc"]

    from contextlib import ExitStack

    import concourse.bass as bass  # noqa: F401
    import concourse.tile as tile
    from concourse import bacc, mybir

    f32 = mybir.dt.float32
    f32r = mybir.dt.float32r
    bf16 = mybir.dt.bfloat16
    AOT = mybir.AluOpType

    nc = bacc.Bacc(None, target_bir_lowering=False)
    x8 = nc.dram_tensor("x8", [BPC, ND, NT], f32, kind="ExternalInput")
    lhbase_d = nc.dram_tensor("lhbase", [2, K, 128, NL], f32r, kind="ExternalInput")
    w2inv24_d = nc.dram_tensor("w2inv24", [24, NT], f32, kind="ExternalInput")
    w2invp8_d = nc.dram_tensor("w2invp8", [8, NT], f32, kind="ExternalInput")
    w2inv1_d = nc.dram_tensor("w2inv1", [1, NT], f32r, kind="ExternalInput")
    einf1_d = nc.dram_tensor("einf1", [128, 1], bf16, kind="ExternalInput")
    wpos_d = nc.dram_tensor("wpos2", [128, 2, NT], bf16, kind="ExternalInput")
    out_d = nc.dram_tensor("out8", [BPC, NP, NT], bf16, kind="ExternalOutput")

    with tile.TileContext(nc) as tc:
        with ExitStack() as ctx:
            persist = ctx.enter_context(tc.tile_pool(name="persist", bufs=1))
            c_pool = ctx.enter_context(tc.tile_pool(name="cmin", bufs=2))
            # [128, NT] half-row tiles: sqrt(j,h) only waits on group h's two
            # matmuls, and 4 bufs give two rows of pipeline slack
            psum_pool = ctx.enter_context(
                tc.tile_pool(name="psum", bufs=4, space="PSUM")
            )

            HT = NT // 2
            # lhsT per group: free layout (m, l), l contiguous for patts DMA
            lh = [persist.tile([K, 128, NL], f32r, name=f"lh{h}") for h in range(2)]
            # rhs split by column half so each matmul waits only on its half
            xwh = [persist.tile([K, HT], f32r, name=f"xw{s}") for s in range(2)]
            w2inv24h = [persist.tile([24, HT], f32, name=f"w2inv24{s}") for s in range(2)]
            w2invp8 = persist.tile([8, NT], f32, name="w2invp8")
            xgh = [persist.tile([24, HT], f32, name=f"xg{s}") for s in range(2)]
            xa8 = persist.tile([8, ND, NT], f32, name="xa8")
            x2sum = persist.tile([8, NT], f32, name="x2sum")
            x2w = persist.tile([8, NT], f32r, name="x2w")
            wpos = persist.tile([128, 2, NT], bf16, name="wpos")
            inf2 = persist.tile([128, NT // 2], f32, name="inf2")
            einf = persist.tile([128, 1], bf16, name="einf")
            E0 = persist.tile([128, NT2 + 1], bf16, name="E0")
            E1 = persist.tile([128, NT2 + 1], bf16, name="E1")
            E = [E0, E1]
            d3r = [persist.tile([128, NT2], f32, name=f"d3_{i}") for i in range(4)]
            oth = persist.tile([128, 2, NT], bf16, name="oth")

            # ---------------- startup ----------------
            # critical path to the first scan: xa8 DMA -> squares -> x2sum
            # adds -> x2w -> xw[24:32] DMA -> matmuls -> sqrts.  Everything
            # else (p2e chain, lhbase, tables) is ordered to stay off it.
            actd = persist.tile([1, 1], f32, name="actd")
            nc.vector.memset(actd[:], 1.0)
            nc.scalar.sqrt(actd[:], actd[:])  # preload the Sqrt ACT table

            xgf = x8.rearrange("b d t -> (b d) t")
            # x DMAs split by t-half so the x2 pipeline starts on half 0.
            # HWDGE descriptor gen costs ~625ns per DMA per queue, so the
            # critical-path DMAs are spread: sync carries half-0 + lhbase,
            # scalar carries xa8 + the tiny xw fills, gpsimd (SWDGE, idle)
            # carries everything half-1 and the constants.
            nc.scalar.dma_start(xa8[:, :, 0:HT], x8[:, :, 0:HT])
            nc.scalar.dma_start(xa8[:, :, HT:NT], x8[:, :, HT:NT])
            nc.scalar.dma_start(xwh[0][32:33, :], w2inv1_d[:, 0:HT])
            nc.scalar.dma_start(w2invp8[:], w2invp8_d[:])
            nc.sync.dma_start(xgh[0][:], xgf[:, 0:HT])
            nc.sync.dma_start(w2inv24h[0][:], w2inv24_d[:, 0:HT])
            nc.sync.dma_start(lh[0][:, :, :], lhbase_d[0])
            nc.sync.dma_start(lh[1][:, :, :], lhbase_d[1])
            nc.gpsimd.dma_start(einf[:], einf1_d[:])
            nc.gpsimd.dma_start(w2inv24h[1][:], w2inv24_d[:, HT:NT])
            nc.gpsimd.dma_start(xgh[1][:], xgf[:, HT:NT])
            nc.gpsimd.dma_start(xwh[1][32:33, :], w2inv1_d[:, HT:NT])

            # rhs (xw) build, pipelined by t-half (half 0 additionally by
            # d-slice): square -> two adds -> w2inv mult -> DMA rows 24:32
            for d in range(3):
                nc.scalar.square(xa8[:, d, 0:HT], xa8[:, d, 0:HT])
            nc.vector.tensor_tensor(
                x2sum[:, 0:HT], xa8[:, 0, 0:HT], xa8[:, 1, 0:HT], op=AOT.add
            )
            nc.vector.tensor_tensor(
                xwh[0][0:24, :], xgh[0][:], w2inv24h[0][:], op=AOT.mult
            )
            nc.vector.tensor_tensor(
                x2sum[:, 0:HT], x2sum[:, 0:HT], xa8[:, 2, 0:HT], op=AOT.add
            )
            nc.vector.tensor_tensor(
                x2w[:, 0:HT], x2sum[:, 0:HT], w2invp8[:, 0:HT], op=AOT.mult
            )
            nc.scalar.dma_start(xwh[0][24:32, :], x2w[:, 0:HT])
            nc.scalar.square(xa8[:, :, HT:NT], xa8[:, :, HT:NT])
            nc.vector.tensor_tensor(
                x2sum[:, HT:NT], xa8[:, 0, HT:NT], xa8[:, 1, HT:NT], op=AOT.add
            )
            nc.vector.tensor_tensor(
                xwh[1][0:24, :], xgh[1][:], w2inv24h[1][:], op=AOT.mult
            )
            nc.vector.tensor_tensor(
                x2sum[:, HT:NT], x2sum[:, HT:NT], xa8[:, 2, HT:NT], op=AOT.add
            )
            nc.vector.tensor_tensor(
                x2w[:, HT:NT], x2sum[:, HT:NT], w2invp8[:, HT:NT], op=AOT.mult
            )
            nc.scalar.dma_start(xwh[1][24:32, :], x2w[:, HT:NT])

            # ---------------- DP state init (DVE fills its idle window) ----
            nc.vector.memset(inf2[:], INF)
            # E1 edge slots; E0's edges are only read via the j==0 stash
            nc.vector.tensor_copy(out=E1[:, 0:1], in_=einf[:])
            nc.vector.tensor_copy(out=E1[:, NT + 1 : NT + 2], in_=einf[:])


            # ---------------- main loop over DP rows ----------------
            for j in range(NL):
                d3 = d3r[j % 4]
                if j == 0:
                    # four independent matmul->sqrt quarter pipelines;
                    # column-half 0 (ready first) for both groups, then half 1
                    for q in range(2):
                        for h in range(2):
                            k0 = 0 if h == 0 else NT + 1
                            psq = psum_pool.tile([128, NT], f32, name="ps")
                            nc.tensor.matmul(
                                psq[:, 0:HT], lh[h][:, :, j], xwh[q][:, :],
                                start=True, stop=True,
                            )
                            nc.scalar.sqrt(
                                d3[:, k0 + q * HT : k0 + (q + 1) * HT],
                                psq[:, 0:HT],
                            )
                else:
                    for h in range(2):
                        k0 = 0 if h == 0 else NT + 1
                        ps = psum_pool.tile([128, NT], f32, name="ps")
                        nc.tensor.matmul(
                            ps[:, 0:HT], lh[h][:, :, j], xwh[0][:, :],
                            start=True, stop=True,
                        )
                        nc.tensor.matmul(
                            ps[:, HT:NT], lh[h][:, :, j], xwh[1][:, :],
                            start=True, stop=True,
                        )
                        nc.scalar.sqrt(d3[:, k0 : k0 + NT], ps[:, :])

                Ecur, Eprev = E[j % 2], E[(j + 1) % 2]
                if j == 0:
                    # chunked cumsum chained via `initial`, following the
                    # quarter sqrts down the pipeline
                    for q in range(2):
                        for h in range(2):
                            lo = (1 if h == 0 else NT + 2) + q * HT
                            k0 = (0 if h == 0 else NT + 1) + q * HT
                            nc.vector.tensor_tensor_scan(
                                out=Ecur[:, lo : lo + HT],
                                data0=inf2[:],
                                data1=d3[:, k0 : k0 + HT],
                                initial=0.0 if q == 0 else Ecur[:, lo - 1 : lo],
                                op0=AOT.min,
                                op1=AOT.add,
                            )
                    # stash E0[t=0] per group into the edge slots so row 1's
                    # shifted-data0 view (cumsum monotonicity shortcut) is
                    # exact at t=0
                    nc.vector.tensor_copy(out=Ecur[:, 0:1], in_=Ecur[:, 1:2])
                    nc.vector.tensor_copy(
                        out=Ecur[:, NT + 1 : NT + 2], in_=Ecur[:, NT + 2 : NT + 3]
                    )
                elif j == 1:
                    # min(E0[t], E0[t-1]) == E0[t-1] by monotonicity; split
                    # per group to chase the sqrts
                    for h in range(2):
                        lo = 1 if h == 0 else NT + 2
                        k0 = 0 if h == 0 else NT + 1
                        nc.vector.tensor_tensor_scan(
                            out=Ecur[:, lo : lo + NT],
                            data0=Eprev[:, k0 : k0 + NT],
                            data1=d3[:, k0 : k0 + NT],
                            initial=INF,
                            op0=AOT.min,
                            op1=AOT.add,
                        )
                    # restore the INF edges for row 2's buffer reuse
                    nc.vector.tensor_copy(out=Eprev[:, 0:1], in_=einf[:])
                    nc.vector.tensor_copy(
                        out=Eprev[:, NT + 1 : NT + 2], in_=einf[:]
                    )
                elif j < NL - 1:
                    # per-group split with order [wminB, wminA, scanB, scanA]:
                    # every op's producer is >=2 DVE ops back, so the
                    # same-engine semaphore round-trips hide behind execution
                    c3 = c_pool.tile([128, NT2], bf16, name="c3")
                    for h in (1, 0):
                        k0 = 0 if h == 0 else NT + 1
                        nc.vector.tensor_tensor(
                            c3[:, k0 : k0 + NT],
                            Eprev[:, k0 + 1 : k0 + 1 + NT],
                            Eprev[:, k0 : k0 + NT],
                            op=AOT.min,
                        )
                    for h in (1, 0):
                        k0 = 0 if h == 0 else NT + 1
                        lo = k0 + 1
                        nc.vector.tensor_tensor_scan(
                            out=Ecur[:, lo : lo + NT],
                            data0=c3[:, k0 : k0 + NT],
                            data1=d3[:, k0 : k0 + NT],
                            initial=INF,
                            op0=AOT.min,
                            op1=AOT.add,
                        )
                else:
                    # last row in quarter chunks: each chunk's rescale +
                    # store overlaps the remaining chunks' scans
                    of = out_d.rearrange("b p t -> (b p) t")
                    engs = [nc.sync, nc.scalar, nc.gpsimd, nc.sync]
                    c3 = c_pool.tile([128, NT2], bf16, name="c3")
                    for hh in (1, 0):
                        k0 = 0 if hh == 0 else NT + 1
                        nc.vector.tensor_tensor(
                            c3[:, k0 : k0 + NT],
                            Eprev[:, k0 + 1 : k0 + 1 + NT],
                            Eprev[:, k0 : k0 + NT],
                            op=AOT.min,
                        )
                    for h in range(2):
                        rows = slice(h * 128, (h + 1) * 128)
                        for q in range(2):
                            lo = (1 if h == 0 else NT + 2) + q * HT
                            k0 = (0 if h == 0 else NT + 1) + q * HT
                            t0 = q * HT
                            nc.vector.tensor_tensor_scan(
                                out=Ecur[:, lo : lo + HT],
                                data0=c3[:, k0 : k0 + HT],
                                data1=d3[:, k0 : k0 + HT],
                                initial=INF if q == 0 else Ecur[:, lo - 1 : lo],
                                op0=AOT.min,
                                op1=AOT.add,
                            )
                            nc.vector.tensor_tensor(
                                oth[:, h, t0 : t0 + HT],
                                Ecur[:, lo : lo + HT],
                                wpos[:, h, t0 : t0 + HT],
                                op=AOT.mult,
                            )
                            engs[2 * h + q].dma_start(
                                of[rows, t0 : t0 + HT], oth[:, h, t0 : t0 + HT]
                            )
                if j == 0:
                    # 1MB table only needed by the output stage; load it
                    # behind the startup DMAs
                    nc.gpsimd.dma_start(wpos[:], wpos_d[:])

    nc.compile()
    _CACHE["nc"] = nc
    return nc


def _in_maps(x, patts):
    W2INV24, W2INVP8, W2INV1, WPOS2, EINF1 = _tables()
    x = np.ascontiguousarray(np.asarray(x, dtype=np.float32))
    patts = np.ascontiguousarray(np.asarray(patts, dtype=np.float32))
    lhb = _lhbase(patts)
    maps = []
    for c in range(NCORES):
        maps.append(
            {
                "x8": np.ascontiguousarray(x[c * BPC : (c + 1) * BPC]),
                "lhbase": lhb,
                "w2inv24": W2INV24,
                "w2invp8": W2INVP8,
                "w2inv1": W2INV1,
                "einf1": EINF1,
                "wpos2": WPOS2,
            }
        )
    return maps


def kernel(x, patts):
    nc = _build()
    from concourse.bass_utils import run_bass_kernel_spmd

    res = run_bass_kernel_spmd(
        nc, _in_maps(x, patts), core_ids=list(range(NCORES))
    )
    _CACHE["last_results"] = res
    out = np.concatenate([r["out8"] for r in res.results], axis=0)
    return out.astype(np.float32)


# revision 40
# speedup vs baseline: 1.0064x; 1.0064x over previous
"""Trainium2 Bass kernel for DTWFeatures.

Problem: x (64,3,1024), patts (32,3,32) -> out (64,32,1024)
  dist[b,p,l,t] = sqrt(max(|x[b,:,t]-patts[p,:,l]|^2, eps))
  DP:  D[l,t] = dist[l,t] + min(D[l-1,t], w*D[l,t-1], w*D[l-1,t-1])
  out[b,p,t] = D[L-1,t]

Strategy (8 cores, data-parallel over batch, 8 batches/core, 256 (b,p)
pairs/core = 2 groups of 128 partitions):
  * Rescale E[l,t] = D[l,t]*w^-(t-SHIFT): removes w from the recurrence.
  * Per DP row the recurrence is one DVE tensor_tensor_scan
    (op0=min, op1=add) with data0 = windowed min of the previous row.
  * Both 128-pair groups are processed by ONE 2049-wide scan per row: a
    boundary element with data1=1e30 blows the scan state up to ~1e30,
    which the next element's min() discards -- an exact chain reset.
  * E rows are stored bf16 (scan state stays fp32 internally), so the
    window-min TensorTensor runs in the DVE 2x_1p mode (2 elem/cycle).
    dist stays fp32.  Validated: L2 rel err ~3e-3 (tolerance 2e-2).
  * dist'^2 comes from a single K=33 fp32r matmul per 512-col chunk
    (fp32r = 1 cycle/row vs fp32's 4): rhs rows = x*-2w2inv (24),
    x2*w2inv (8), w2inv (1); lhsT = block-diag patts / batch indicators /
    p2+eps, one lhsT per group.  EPS=1.5e-2 floors d^2 against fp32r
    cancellation noise (HW sqrt(neg)=NaN).
  * ScalarE applies sqrt PSUM->SBUF; DVE is the bottleneck engine at
    ~105us busy (32 scans of 2049 + 30 bf16 window-mins).
"""

import os
import sys

if "/opt/trn_rl_repo" not in sys.path:
    sys.path.insert(0, "/opt/trn_rl_repo")
# the device path runs through jax's axon PJRT backend; make sure a
# harness-pinned JAX_PLATFORMS doesn't hide it (no-op if jax is already up)
if "jax" not in sys.modules and "axon" not in os.environ.get(
    "JAX_PLATFORMS", "axon"
):
    os.environ["JAX_PLATFORMS"] = "axon," + os.environ["JAX_PLATFORMS"]

import numpy as np

NB, ND, NP, NL, NT = 64, 3, 32, 32, 1024   # batch, xdim, n_patts, l_patts, T
NCORES = 8
BPC = NB // NCORES                     # 8 batches per core
RHO = 0.1
W = RHO ** (1.0 / NL)
SHIFT = 512.0
EPS = 1.5e-2                           # floors d^2 against fp32r noise
INF = 1.0e30
LARGE = 1.0e30                         # scan chain-reset boundary value
K = 33                                 # matmul contraction rows
NT2 = 2 * NT + 1                       # merged scan length (1024+1+1024)

_CACHE = {}


def _tables():
    if "tables" not in _CACHE:
        import ml_dtypes

        t = np.arange(NT, dtype=np.float64)
        w2inv = (W ** (-2.0 * (t - SHIFT))).astype(np.float32)
        wpos = (W ** (t - SHIFT)).astype(np.float32)
        W2INV24 = np.ascontiguousarray(np.tile(w2inv[None, :], (24, 1)) * -2.0)
        W2INVP8 = np.ascontiguousarray(np.tile(w2inv[None, :], (8, 1)))
        W2INV1 = np.ascontiguousarray(w2inv[None, :])
        WPOS2 = np.ascontiguousarray(np.tile(wpos[None, None, :], (128, 2, 1))).astype(ml_dtypes.bfloat16)
        EINF1 = np.full((128, 1), INF, np.float32).astype(ml_dtypes.bfloat16)
        _CACHE["tables"] = (W2INV24, W2INVP8, W2INV1, WPOS2, EINF1)
    return _CACHE["tables"]


def _lhbase(patts):
    """Full stationary lhsT per group: block-diagonal raw patts (rows 0:24,
    the -2 scale lives in the rhs), batch indicators (rows 24:32), and the
    p^2+eps row (row 32, paired with the rhs w2inv row)."""
    lhb = np.zeros((2, K, 128, NL), np.float32)
    pdl = np.transpose(patts, (1, 0, 2))  # (d, p, l)
    p2e = np.einsum("pdl,pdl->pl", patts, patts) + EPS
    for h in range(2):
        for bl in range(4):
            r = 12 * h + 3 * bl
            lhb[h, r : r + 3, 32 * bl : 32 * (bl + 1), :] = pdl
            lhb[h, 24 + 4 * h + bl, 32 * bl : 32 * (bl + 1), :] = 1.0
            lhb[h, 32, 32 * bl : 32 * (bl + 1), :] = p2e
    return lhb


def _build():
    if "nc" in _CACHE:
        return _CACHE["nc"]

    from contextlib import ExitStack

    import concourse.bass as bass  # noqa: F401
    import concourse.tile as tile
    from concourse import bacc, mybir

    f32 = mybir.dt.float32
    f32r = mybir.dt.float32r
    bf16 = mybir.dt.bfloat16
    AOT = mybir.AluOpType

    nc = bacc.Bacc(None, target_bir_lowering=False)
    x8 = nc.dram_tensor("x8", [BPC, ND, NT], f32, kind="ExternalInput")
    lhbase_d = nc.dram_tensor("lhbase", [2, K, 128, NL], f32r, kind="ExternalInput")
    w2inv24_d = nc.dram_tensor("w2inv24", [24, NT], f32, kind="ExternalInput")
    w2invp8_d = nc.dram_tensor("w2invp8", [8, NT], f32, kind="ExternalInput")
    w2inv1_d = nc.dram_tensor("w2inv1", [1, NT], f32r, kind="ExternalInput")
    einf1_d = nc.dram_tensor("einf1", [128, 1], bf16, kind="ExternalInput")
    wpos_d = nc.dram_tensor("wpos2", [128, 2, NT], bf16, kind="ExternalInput")
    out_d = nc.dram_tensor("out8", [BPC, NP, NT], bf16, kind="ExternalOutput")

    with tile.TileContext(nc) as tc:
        with ExitStack() as ctx:
            persist = ctx.enter_context(tc.tile_pool(name="persist", bufs=1))
            c_pool = ctx.enter_context(tc.tile_pool(name="cmin", bufs=2))
            # [128, NT] half-row tiles: sqrt(j,h) only waits on group h's two
            # matmuls, and 4 bufs give two rows of pipeline slack
            psum_pool = ctx.enter_context(
                tc.tile_pool(name="psum", bufs=4, space="PSUM")
            )

            HT = NT // 2
            # lhsT per group: free layout (m, l), l contiguous for patts DMA
            lh = [persist.tile([K, 128, NL], f32r, name=f"lh{h}") for h in range(2)]
            # rhs split by column half so each matmul waits only on its half
            xwh = [persist.tile([K, HT], f32r, name=f"xw{s}") for s in range(2)]
            w2inv24h = [persist.tile([24, HT], f32, name=f"w2inv24{s}") for s in range(2)]
            w2invp8 = persist.tile([8, NT], f32, name="w2invp8")
            xgh = [persist.tile([24, HT], f32, name=f"xg{s}") for s in range(2)]
            xa8 = persist.tile([8, ND, NT], f32, name="xa8")
            x2sum = persist.tile([8, NT], f32, name="x2sum")
            x2w = persist.tile([8, NT], f32r, name="x2w")
            wpos = persist.tile([128, 2, NT], bf16, name="wpos")
            inf2 = persist.tile([128, NT // 2], f32, name="inf2")
            einf = persist.tile([128, 1], bf16, name="einf")
            E0 = persist.tile([128, NT2 + 1], bf16, name="E0")
            E1 = persist.tile([128, NT2 + 1], bf16, name="E1")
            E = [E0, E1]
            d3r = [persist.tile([128, NT2], f32, name=f"d3_{i}") for i in range(4)]
            oth = persist.tile([128, 2, NT], bf16, name="oth")

            # ---------------- startup ----------------
            # critical path to the first scan: xa8 DMA -> squares -> x2sum
            # adds -> x2w -> xw[24:32] DMA -> matmuls -> sqrts.  Everything
            # else (p2e chain, lhbase, tables) is ordered to stay off it.
            actd = persist.tile([1, 1], f32, name="actd")
            nc.vector.memset(actd[:], 1.0)
            nc.scalar.sqrt(actd[:], actd[:])  # preload the Sqrt ACT table

            xgf = x8.rearrange("b d t -> (b d) t")
            # x DMAs split by t-half so the x2 pipeline starts on half 0.
            # HWDGE descriptor gen costs ~625ns per DMA per queue, so the
            # critical-path DMAs are spread: sync carries half-0 + lhbase,
            # scalar carries xa8 + the tiny xw fills, gpsimd (SWDGE, idle)
            # carries everything half-1 and the constants.
            nc.scalar.dma_start(xa8[:, :, 0:HT], x8[:, :, 0:HT])
            nc.scalar.dma_start(xa8[:, :, HT:NT], x8[:, :, HT:NT])
            nc.scalar.dma_start(xwh[0][32:33, :], w2inv1_d[:, 0:HT])
            nc.scalar.dma_start(w2invp8[:], w2invp8_d[:])
            nc.sync.dma_start(xgh[0][:], xgf[:, 0:HT])
            nc.sync.dma_start(w2inv24h[0][:], w2inv24_d[:, 0:HT])
            nc.sync.dma_start(xwh[1][32:33, :], w2inv1_d[:, HT:NT])
            nc.sync.dma_start(lh[0][:, :, :], lhbase_d[0])
            nc.sync.dma_start(xgh[1][:], xgf[:, HT:NT])
            nc.sync.dma_start(w2inv24h[1][:], w2inv24_d[:, HT:NT])
            nc.sync.dma_start(lh[1][:, :, :], lhbase_d[1])
            nc.gpsimd.dma_start(einf[:], einf1_d[:])

            # rhs (xw) build, pipelined by t-half:
            # square -> two adds -> w2inv mult -> DMA into xw rows 24:32
            for s in range(2):
                ts0, ts1 = s * HT, (s + 1) * HT
                nc.scalar.square(xa8[:, :, ts0:ts1], xa8[:, :, ts0:ts1])
                nc.vector.tensor_tensor(
                    xwh[s][0:24, :], xgh[s][:], w2inv24h[s][:], op=AOT.mult
                )
                nc.vector.tensor_tensor(
                    x2sum[:, ts0:ts1], xa8[:, 0, ts0:ts1], xa8[:, 1, ts0:ts1],
                    op=AOT.add,
                )
                nc.vector.tensor_tensor(
                    x2sum[:, ts0:ts1], x2sum[:, ts0:ts1], xa8[:, 2, ts0:ts1],
                    op=AOT.add,
                )
                nc.vector.tensor_tensor(
                    x2w[:, ts0:ts1], x2sum[:, ts0:ts1], w2invp8[:, ts0:ts1],
                    op=AOT.mult,
                )
                nc.scalar.dma_start(xwh[s][24:32, :], x2w[:, ts0:ts1])

            # ---------------- DP state init (DVE fills its idle window) ----
            nc.vector.memset(inf2[:], INF)
            # E1 edge slots; E0's edges are only read via the j==0 stash
            nc.vector.tensor_copy(out=E1[:, 0:1], in_=einf[:])
            nc.vector.tensor_copy(out=E1[:, NT + 1 : NT + 2], in_=einf[:])


            # ---------------- main loop over DP rows ----------------
            for j in range(NL):
                d3 = d3r[j % 4]
                if j == 0:
                    # four independent matmul->sqrt quarter pipelines;
                    # column-half 0 (ready first) for both groups, then half 1
                    for q in range(2):
                        for h in range(2):
                            k0 = 0 if h == 0 else NT + 1
                            psq = psum_pool.tile([128, NT], f32, name="ps")
                            nc.tensor.matmul(
                                psq[:, 0:HT], lh[h][:, :, j], xwh[q][:, :],
                                start=True, stop=True,
                            )
                            nc.scalar.sqrt(
                                d3[:, k0 + q * HT : k0 + (q + 1) * HT],
                                psq[:, 0:HT],
                            )
                else:
                    for h in range(2):
                        k0 = 0 if h == 0 else NT + 1
                        ps = psum_pool.tile([128, NT], f32, name="ps")
                        nc.tensor.matmul(
                            ps[:, 0:HT], lh[h][:, :, j], xwh[0][:, :],
                            start=True, stop=True,
                        )
                        nc.tensor.matmul(
                            ps[:, HT:NT], lh[h][:, :, j], xwh[1][:, :],
                            start=True, stop=True,
                        )
                        nc.scalar.sqrt(d3[:, k0 : k0 + NT], ps[:, :])

                Ecur, Eprev = E[j % 2], E[(j + 1) % 2]
                if j == 0:
                    # chunked cumsum chained via `initial`, following the
                    # quarter sqrts down the pipeline
                    for q in range(2):
                        for h in range(2):
                            lo = (1 if h == 0 else NT + 2) + q * HT
                            k0 = (0 if h == 0 else NT + 1) + q * HT
                            nc.vector.tensor_tensor_scan(
                                out=Ecur[:, lo : lo + HT],
                                data0=inf2[:],
                                data1=d3[:, k0 : k0 + HT],
                                initial=0.0 if q == 0 else Ecur[:, lo - 1 : lo],
                                op0=AOT.min,
                                op1=AOT.add,
                            )
                    # stash E0[t=0] per group into the edge slots so row 1's
                    # shifted-data0 view (cumsum monotonicity shortcut) is
                    # exact at t=0
                    nc.vector.tensor_copy(out=Ecur[:, 0:1], in_=Ecur[:, 1:2])
                    nc.vector.tensor_copy(
                        out=Ecur[:, NT + 1 : NT + 2], in_=Ecur[:, NT + 2 : NT + 3]
                    )
                elif j == 1:
                    # min(E0[t], E0[t-1]) == E0[t-1] by monotonicity; split
                    # per group to chase the sqrts
                    for h in range(2):
                        lo = 1 if h == 0 else NT + 2
                        k0 = 0 if h == 0 else NT + 1
                        nc.vector.tensor_tensor_scan(
                            out=Ecur[:, lo : lo + NT],
                            data0=Eprev[:, k0 : k0 + NT],
                            data1=d3[:, k0 : k0 + NT],
                            initial=INF,
                            op0=AOT.min,
                            op1=AOT.add,
                        )
                    # restore the INF edges for row 2's buffer reuse
                    nc.vector.tensor_copy(out=Eprev[:, 0:1], in_=einf[:])
                    nc.vector.tensor_copy(
                        out=Eprev[:, NT + 1 : NT + 2], in_=einf[:]
                    )
                elif j < NL - 1:
                    # per-group split with order [wminB, wminA, scanB, scanA]:
                    # every op's producer is >=2 DVE ops back, so the
                    # same-engine semaphore round-trips hide behind execution
                    c3 = c_pool.tile([128, NT2], bf16, name="c3")
                    for h in (1, 0):
                        k0 = 0 if h == 0 else NT + 1
                        nc.vector.tensor_tensor(
                            c3[:, k0 : k0 + NT],
                            Eprev[:, k0 + 1 : k0 + 1 + NT],
                            Eprev[:, k0 : k0 + NT],
                            op=AOT.min,
                        )
                    for h in (1, 0):
                        k0 = 0 if h == 0 else NT + 1
                        lo = k0 + 1
                        nc.vector.tensor_tensor_scan(
                            out=Ecur[:, lo : lo + NT],
                            data0=c3[:, k0 : k0 + NT],
                            data1=d3[:, k0 : k0 + NT],
                            initial=INF,
                            op0=AOT.min,
                            op1=AOT.add,
                        )
                else:
                    # last row in quarter chunks: each chunk's rescale +
                    # store overlaps the remaining chunks' scans
                    of = out_d.rearrange("b p t -> (b p) t")
                    engs = [nc.sync, nc.scalar, nc.gpsimd, nc.sync]
                    c3 = c_pool.tile([128, NT2], bf16, name="c3")
                    for hh in (1, 0):
                        k0 = 0 if hh == 0 else NT + 1
                        nc.vector.tensor_tensor(
                            c3[:, k0 : k0 + NT],
                            Eprev[:, k0 + 1 : k0 + 1 + NT],
                            Eprev[:, k0 : k0 + NT],
                            op=AOT.min,
                        )
                    for h in range(2):
                        rows = slice(h * 128, (h + 1) * 128)
                        for q in range(2):
                            lo = (1 if h == 0 else NT + 2) + q * HT
                            k0 = (0 if h == 0 else NT + 1) + q * HT
                            t0 = q * HT
                            nc.vector.tensor_tensor_scan(
                                out=Ecur[:, lo : lo + HT],
                                data0=c3[:, k0 : k0 + HT],
                                data1=d3[:, k0 : k0 + HT],
                                initial=INF if q == 0 else Ecur[:, lo - 1 : lo],
                                op0=AOT.min,
                                op1=AOT.add,
                            )
                            nc.vector.tensor_tensor(
                                oth[:, h, t0 : t0 + HT],
                                Ecur[:, lo : lo + HT],
                                wpos[:, h, t0 : t0 + HT],
                                op=AOT.mult,
                            )
                            engs[2 * h + q].dma_start(
                                of[rows, t0 : t0 + HT], oth[:, h, t0 : t0 + HT]
                            )
                if j == 0:
                    # 1MB table only needed by the output stage; load it
                    # behind the startup DMAs
                    nc.gpsimd.dma_start(wpos[:], wpos_d[:])

    nc.compile()
    _CACHE["nc"] = nc
    return nc


def _in_maps(x, patts):
    W2INV24, W2INVP8, W2INV1, WPOS2, EINF1 = _tables()
    x = np.ascontiguousarray(np.asarray(x, dtype=np.float32))
    patts = np.ascontiguousarray(np.asarray(patts, dtype=np.float32))
    lhb = _lhbase(patts)
    maps = []
    for c in range(NCORES):
        maps.append(
            {
                "x8": np.ascontiguousarray(x[c * BPC : (c + 1) * BPC]),
                "lhbase": lhb,
                "w2inv24": W2INV24,
                "w2invp8": W2INVP8,
                "w2inv1": W2INV1,
                "einf1": EINF1,
                "wpos2": WPOS2,
            }
        )
    return maps


def kernel(x, patts):
    nc = _build()
    from concourse.bass_utils import run_bass_kernel_spmd

    res = run_bass_kernel_spmd(
        nc, _in_maps(x, patts), core_ids=list(range(NCORES))
    )
    _CACHE["last_results"] = res
    out = np.concatenate([r["out8"] for r in res.results], axis=0)
    return out.astype(np.float32)


# revision 41
# speedup vs baseline: 1.0217x; 1.0153x over previous
"""Trainium2 Bass kernel for DTWFeatures.

Problem: x (64,3,1024), patts (32,3,32) -> out (64,32,1024)
  dist[b,p,l,t] = sqrt(max(|x[b,:,t]-patts[p,:,l]|^2, eps))
  DP:  D[l,t] = dist[l,t] + min(D[l-1,t], w*D[l,t-1], w*D[l-1,t-1])
  out[b,p,t] = D[L-1,t]

Strategy (8 cores, data-parallel over batch, 8 batches/core, 256 (b,p)
pairs/core = 2 groups of 128 partitions):
  * Rescale E[l,t] = D[l,t]*w^-(t-SHIFT): removes w from the recurrence.
  * Per DP row the recurrence is one DVE tensor_tensor_scan
    (op0=min, op1=add) with data0 = windowed min of the previous row.
  * Both 128-pair groups are processed by ONE 2049-wide scan per row: a
    boundary element with data1=1e30 blows the scan state up to ~1e30,
    which the next element's min() discards -- an exact chain reset.
  * E rows are stored bf16 (scan state stays fp32 internally), so the
    window-min TensorTensor runs in the DVE 2x_1p mode (2 elem/cycle).
    dist stays fp32.  Validated: L2 rel err ~3e-3 (tolerance 2e-2).
  * dist'^2 comes from a single K=33 fp32r matmul per 512-col chunk
    (fp32r = 1 cycle/row vs fp32's 4): rhs rows = x*-2w2inv (24),
    x2*w2inv (8), w2inv (1); lhsT = block-diag patts / batch indicators /
    p2+eps, one lhsT per group.  EPS=1.5e-2 floors d^2 against fp32r
    cancellation noise (HW sqrt(neg)=NaN).
  * ScalarE applies sqrt PSUM->SBUF; DVE is the bottleneck engine at
    ~105us busy (32 scans of 2049 + 30 bf16 window-mins).
"""

import os
import sys

if "/opt/trn_rl_repo" not in sys.path:
    sys.path.insert(0, "/opt/trn_rl_repo")
# the device path runs through jax's axon PJRT backend; make sure a
# harness-pinned JAX_PLATFORMS doesn't hide it (no-op if jax is already up)
if "jax" not in sys.modules and "axon" not in os.environ.get(
    "JAX_PLATFORMS", "axon"
):
    os.environ["JAX_PLATFORMS"] = "axon," + os.environ["JAX_PLATFORMS"]

import numpy as np

NB, ND, NP, NL, NT = 64, 3, 32, 32, 1024   # batch, xdim, n_patts, l_patts, T
NCORES = 8
BPC = NB // NCORES                     # 8 batches per core
RHO = 0.1
W = RHO ** (1.0 / NL)
SHIFT = 512.0
EPS = 1.5e-2                           # floors d^2 against fp32r noise
INF = 1.0e30
LARGE = 1.0e30                         # scan chain-reset boundary value
K = 33                                 # matmul contraction rows
NT2 = 2 * NT + 1                       # merged scan length (1024+1+1024)

_CACHE = {}


def _tables():
    if "tables" not in _CACHE:
        import ml_dtypes

        t = np.arange(NT, dtype=np.float64)
        w2inv = (W ** (-2.0 * (t - SHIFT))).astype(np.float32)
        wpos = (W ** (t - SHIFT)).astype(np.float32)
        W2INV24 = np.ascontiguousarray(np.tile(w2inv[None, :], (24, 1)) * -2.0)
        W2INVP8 = np.ascontiguousarray(np.tile(w2inv[None, :], (8, 1)))
        W2INV1 = np.ascontiguousarray(w2inv[None, :])
        WPOS2 = np.ascontiguousarray(np.tile(wpos[None, None, :], (128, 2, 1))).astype(ml_dtypes.bfloat16)
        EINF1 = np.full((128, 1), INF, np.float32).astype(ml_dtypes.bfloat16)
        _CACHE["tables"] = (W2INV24, W2INVP8, W2INV1, WPOS2, EINF1)
    return _CACHE["tables"]


def _lhbase(patts):
    """Full stationary lhsT per group: block-diagonal raw patts (rows 0:24,
    the -2 scale lives in the rhs), batch indicators (rows 24:32), and the
    p^2+eps row (row 32, paired with the rhs w2inv row)."""
    lhb = np.zeros((2, K, 128, NL), np.float32)
    pdl = np.transpose(patts, (1, 0, 2))  # (d, p, l)
    p2e = np.einsum("pdl,pdl->pl", patts, patts) + EPS
    for h in range(2):
        for bl in range(4):
            r = 12 * h + 3 * bl
            lhb[h, r : r + 3, 32 * bl : 32 * (bl + 1), :] = pdl
            lhb[h, 24 + 4 * h + bl, 32 * bl : 32 * (bl + 1), :] = 1.0
            lhb[h, 32, 32 * bl : 32 * (bl + 1), :] = p2e
    return lhb


def _build():
    if "nc" in _CACHE:
        return _CACHE["nc"]

    from contextlib import ExitStack

    import concourse.bass as bass  # noqa: F401
    import concourse.tile as tile
    from concourse import bacc, mybir

    f32 = mybir.dt.float32
    f32r = mybir.dt.float32r
    bf16 = mybir.dt.bfloat16
    AOT = mybir.AluOpType

    nc = bacc.Bacc(None, target_bir_lowering=False)
    x8 = nc.dram_tensor("x8", [BPC, ND, NT], f32, kind="ExternalInput")
    lhbase_d = nc.dram_tensor("lhbase", [2, K, 128, NL], f32r, kind="ExternalInput")
    w2inv24_d = nc.dram_tensor("w2inv24", [24, NT], f32, kind="ExternalInput")
    w2invp8_d = nc.dram_tensor("w2invp8", [8, NT], f32, kind="ExternalInput")
    w2inv1_d = nc.dram_tensor("w2inv1", [1, NT], f32r, kind="ExternalInput")
    einf1_d = nc.dram_tensor("einf1", [128, 1], bf16, kind="ExternalInput")
    wpos_d = nc.dram_tensor("wpos2", [128, 2, NT], bf16, kind="ExternalInput")
    out_d = nc.dram_tensor("out8", [BPC, NP, NT], bf16, kind="ExternalOutput")

    with tile.TileContext(nc) as tc:
        with ExitStack() as ctx:
            persist = ctx.enter_context(tc.tile_pool(name="persist", bufs=1))
            c_pool = ctx.enter_context(tc.tile_pool(name="cmin", bufs=2))
            # [128, NT] half-row tiles: sqrt(j,h) only waits on group h's two
            # matmuls, and 4 bufs give two rows of pipeline slack
            psum_pool = ctx.enter_context(
                tc.tile_pool(name="psum", bufs=4, space="PSUM")
            )

            HT = NT // 2
            # lhsT per group: free layout (m, l), l contiguous for patts DMA
            lh = [persist.tile([K, 128, NL], f32r, name=f"lh{h}") for h in range(2)]
            # rhs split by column half so each matmul waits only on its half
            xwh = [persist.tile([K, HT], f32r, name=f"xw{s}") for s in range(2)]
            w2inv24h = [persist.tile([24, HT], f32, name=f"w2inv24{s}") for s in range(2)]
            w2invp8 = persist.tile([8, NT], f32, name="w2invp8")
            xgh = [persist.tile([24, HT], f32, name=f"xg{s}") for s in range(2)]
            xa8 = persist.tile([8, ND, NT], f32, name="xa8")
            x2sum = persist.tile([8, NT], f32, name="x2sum")
            x2w = persist.tile([8, NT], f32r, name="x2w")
            wpos = persist.tile([128, 2, NT], bf16, name="wpos")
            inf2 = persist.tile([128, NT // 2], f32, name="inf2")
            einf = persist.tile([128, 1], bf16, name="einf")
            E0 = persist.tile([128, NT2 + 1], bf16, name="E0")
            E1 = persist.tile([128, NT2 + 1], bf16, name="E1")
            E = [E0, E1]
            d3r = [persist.tile([128, NT2], f32, name=f"d3_{i}") for i in range(4)]
            oth = persist.tile([128, 2, NT], bf16, name="oth")

            # ---------------- startup ----------------
            # critical path to the first scan: xa8 DMA -> squares -> x2sum
            # adds -> x2w -> xw[24:32] DMA -> matmuls -> sqrts.  Everything
            # else (p2e chain, lhbase, tables) is ordered to stay off it.
            actd = persist.tile([1, 1], f32, name="actd")
            nc.vector.memset(actd[:], 1.0)
            nc.scalar.sqrt(actd[:], actd[:])  # preload the Sqrt ACT table

            xgf = x8.rearrange("b d t -> (b d) t")
            # x DMAs split by t-half so the x2 pipeline starts on half 0.
            # HWDGE descriptor gen costs ~625ns per DMA per queue, so the
            # critical-path DMAs are spread: sync carries half-0 + lhbase,
            # scalar carries xa8 + the tiny xw fills, gpsimd (SWDGE, idle)
            # carries everything half-1 and the constants.
            nc.scalar.dma_start(xa8[:, :, 0:HT], x8[:, :, 0:HT])
            nc.scalar.dma_start(xa8[:, :, HT:NT], x8[:, :, HT:NT])
            nc.sync.dma_start(xgh[0][:], xgf[:, 0:HT])
            nc.sync.dma_start(w2inv24h[0][:], w2inv24_d[:, 0:HT])
            nc.sync.dma_start(w2invp8[:], w2invp8_d[:])
            nc.sync.dma_start(xwh[0][32:33, :], w2inv1_d[:, 0:HT])
            nc.sync.dma_start(xwh[1][32:33, :], w2inv1_d[:, HT:NT])
            nc.sync.dma_start(lh[0][:, :, :], lhbase_d[0])
            nc.sync.dma_start(xgh[1][:], xgf[:, HT:NT])
            nc.sync.dma_start(w2inv24h[1][:], w2inv24_d[:, HT:NT])
            nc.sync.dma_start(lh[1][:, :, :], lhbase_d[1])
            nc.gpsimd.dma_start(einf[:], einf1_d[:])

            # rhs (xw) build, pipelined by t-half:
            # square -> two adds -> w2inv mult -> DMA into xw rows 24:32
            for s in range(2):
                ts0, ts1 = s * HT, (s + 1) * HT
                nc.scalar.square(xa8[:, :, ts0:ts1], xa8[:, :, ts0:ts1])
                nc.vector.tensor_tensor(
                    xwh[s][0:24, :], xgh[s][:], w2inv24h[s][:], op=AOT.mult
                )
                nc.vector.tensor_tensor(
                    x2sum[:, ts0:ts1], xa8[:, 0, ts0:ts1], xa8[:, 1, ts0:ts1],
                    op=AOT.add,
                )
                nc.vector.tensor_tensor(
                    x2sum[:, ts0:ts1], x2sum[:, ts0:ts1], xa8[:, 2, ts0:ts1],
                    op=AOT.add,
                )
                nc.vector.tensor_tensor(
                    x2w[:, ts0:ts1], x2sum[:, ts0:ts1], w2invp8[:, ts0:ts1],
                    op=AOT.mult,
                )
                nc.scalar.dma_start(xwh[s][24:32, :], x2w[:, ts0:ts1])

            # ---------------- DP state init (DVE fills its idle window) ----
            nc.vector.memset(inf2[:], INF)
            # E1 edge slots; E0's edges are only read via the j==0 stash
            nc.vector.tensor_copy(out=E1[:, 0:1], in_=einf[:])
            nc.vector.tensor_copy(out=E1[:, NT + 1 : NT + 2], in_=einf[:])


            # ---------------- main loop over DP rows ----------------
            for j in range(NL):
                d3 = d3r[j % 4]
                if j == 0:
                    # four independent matmul->sqrt quarter pipelines;
                    # column-half 0 (ready first) for both groups, then half 1
                    for q in range(2):
                        for h in range(2):
                            k0 = 0 if h == 0 else NT + 1
                            psq = psum_pool.tile([128, NT], f32, name="ps")
                            nc.tensor.matmul(
                                psq[:, 0:HT], lh[h][:, :, j], xwh[q][:, :],
                                start=True, stop=True,
                            )
                            nc.scalar.sqrt(
                                d3[:, k0 + q * HT : k0 + (q + 1) * HT],
                                psq[:, 0:HT],
                            )
                else:
                    for h in range(2):
                        k0 = 0 if h == 0 else NT + 1
                        ps = psum_pool.tile([128, NT], f32, name="ps")
                        nc.tensor.matmul(
                            ps[:, 0:HT], lh[h][:, :, j], xwh[0][:, :],
                            start=True, stop=True,
                        )
                        nc.tensor.matmul(
                            ps[:, HT:NT], lh[h][:, :, j], xwh[1][:, :],
                            start=True, stop=True,
                        )
                        nc.scalar.sqrt(d3[:, k0 : k0 + NT], ps[:, :])

                Ecur, Eprev = E[j % 2], E[(j + 1) % 2]
                if j == 0:
                    # chunked cumsum chained via `initial`, following the
                    # quarter sqrts down the pipeline
                    for q in range(2):
                        for h in range(2):
                            lo = (1 if h == 0 else NT + 2) + q * HT
                            k0 = (0 if h == 0 else NT + 1) + q * HT
                            nc.vector.tensor_tensor_scan(
                                out=Ecur[:, lo : lo + HT],
                                data0=inf2[:],
                                data1=d3[:, k0 : k0 + HT],
                                initial=0.0 if q == 0 else Ecur[:, lo - 1 : lo],
                                op0=AOT.min,
                                op1=AOT.add,
                            )
                    # stash E0[t=0] per group into the edge slots so row 1's
                    # shifted-data0 view (cumsum monotonicity shortcut) is
                    # exact at t=0
                    nc.vector.tensor_copy(out=Ecur[:, 0:1], in_=Ecur[:, 1:2])
                    nc.vector.tensor_copy(
                        out=Ecur[:, NT + 1 : NT + 2], in_=Ecur[:, NT + 2 : NT + 3]
                    )
                elif j == 1:
                    # min(E0[t], E0[t-1]) == E0[t-1] by monotonicity; split
                    # per group to chase the sqrts
                    for h in range(2):
                        lo = 1 if h == 0 else NT + 2
                        k0 = 0 if h == 0 else NT + 1
                        nc.vector.tensor_tensor_scan(
                            out=Ecur[:, lo : lo + NT],
                            data0=Eprev[:, k0 : k0 + NT],
                            data1=d3[:, k0 : k0 + NT],
                            initial=INF,
                            op0=AOT.min,
                            op1=AOT.add,
                        )
                    # restore the INF edges for row 2's buffer reuse
                    nc.vector.tensor_copy(out=Eprev[:, 0:1], in_=einf[:])
                    nc.vector.tensor_copy(
                        out=Eprev[:, NT + 1 : NT + 2], in_=einf[:]
                    )
                elif j < NL - 1:
                    # per-group split with order [wminB, wminA, scanB, scanA]:
                    # every op's producer is >=2 DVE ops back, so the
                    # same-engine semaphore round-trips hide behind execution
                    c3 = c_pool.tile([128, NT2], bf16, name="c3")
                    for h in (1, 0):
                        k0 = 0 if h == 0 else NT + 1
                        nc.vector.tensor_tensor(
                            c3[:, k0 : k0 + NT],
                            Eprev[:, k0 + 1 : k0 + 1 + NT],
                            Eprev[:, k0 : k0 + NT],
                            op=AOT.min,
                        )
                    for h in (1, 0):
                        k0 = 0 if h == 0 else NT + 1
                        lo = k0 + 1
                        nc.vector.tensor_tensor_scan(
                            out=Ecur[:, lo : lo + NT],
                            data0=c3[:, k0 : k0 + NT],
                            data1=d3[:, k0 : k0 + NT],
                            initial=INF,
                            op0=AOT.min,
                            op1=AOT.add,
                        )
                else:
                    # last row in quarter chunks: each chunk's rescale +
                    # store overlaps the remaining chunks' scans
                    of = out_d.rearrange("b p t -> (b p) t")
                    engs = [nc.sync, nc.scalar, nc.gpsimd, nc.sync]
                    c3 = c_pool.tile([128, NT2], bf16, name="c3")
                    for hh in (1, 0):
                        k0 = 0 if hh == 0 else NT + 1
                        nc.vector.tensor_tensor(
                            c3[:, k0 : k0 + NT],
                            Eprev[:, k0 + 1 : k0 + 1 + NT],
                            Eprev[:, k0 : k0 + NT],
                            op=AOT.min,
                        )
                    for h in range(2):
                        rows = slice(h * 128, (h + 1) * 128)
                        for q in range(2):
                            lo = (1 if h == 0 else NT + 2) + q * HT
                            k0 = (0 if h == 0 else NT + 1) + q * HT
                            t0 = q * HT
                            nc.vector.tensor_tensor_scan(
                                out=Ecur[:, lo : lo + HT],
                                data0=c3[:, k0 : k0 + HT],
                                data1=d3[:, k0 : k0 + HT],
                                initial=INF if q == 0 else Ecur[:, lo - 1 : lo],
                                op0=AOT.min,
                                op1=AOT.add,
                            )
                            nc.vector.tensor_tensor(
                                oth[:, h, t0 : t0 + HT],
                                Ecur[:, lo : lo + HT],
                                wpos[:, h, t0 : t0 + HT],
                                op=AOT.mult,
                            )
                            engs[2 * h + q].dma_start(
                                of[rows, t0 : t0 + HT], oth[:, h, t0 : t0 + HT]
                            )
                if j == 0:
                    # 1MB table only needed by the output stage; load it
                    # behind the startup DMAs
                    nc.gpsimd.dma_start(wpos[:], wpos_d[:])

    nc.compile()
    _CACHE["nc"] = nc
    return nc


def _in_maps(x, patts):
    W2INV24, W2INVP8, W2INV1, WPOS2, EINF1 = _tables()
    x = np.ascontiguousarray(np.asarray(x, dtype=np.float32))
    patts = np.ascontiguousarray(np.asarray(patts, dtype=np.float32))
    lhb = _lhbase(patts)
    maps = []
    for c in range(NCORES):
        maps.append(
            {
                "x8": np.ascontiguousarray(x[c * BPC : (c + 1) * BPC]),
                "lhbase": lhb,
                "w2inv24": W2INV24,
                "w2invp8": W2INVP8,
                "w2inv1": W2INV1,
                "einf1": EINF1,
                "wpos2": WPOS2,
            }
        )
    return maps


def kernel(x, patts):
    nc = _build()
    from concourse.bass_utils import run_bass_kernel_spmd

    res = run_bass_kernel_spmd(
        nc, _in_maps(x, patts), core_ids=list(range(NCORES))
    )
    _CACHE["last_results"] = res
    out = np.concatenate([r["out8"] for r in res.results], axis=0)
    return out.astype(np.float32)


# revision 43
# speedup vs baseline: 1.0237x; 1.0020x over previous
"""Trainium2 Bass kernel for DTWFeatures.

Problem: x (64,3,1024), patts (32,3,32) -> out (64,32,1024)
  dist[b,p,l,t] = sqrt(max(|x[b,:,t]-patts[p,:,l]|^2, eps))
  DP:  D[l,t] = dist[l,t] + min(D[l-1,t], w*D[l,t-1], w*D[l-1,t-1])
  out[b,p,t] = D[L-1,t]

Strategy (8 cores, data-parallel over batch, 8 batches/core, 256 (b,p)
pairs/core = 2 groups of 128 partitions):
  * Rescale E[l,t] = D[l,t]*w^-(t-SHIFT): removes w from the recurrence.
  * Per DP row the recurrence is one DVE tensor_tensor_scan
    (op0=min, op1=add) with data0 = windowed min of the previous row.
  * Both 128-pair groups are processed by ONE 2049-wide scan per row: a
    boundary element with data1=1e30 blows the scan state up to ~1e30,
    which the next element's min() discards -- an exact chain reset.
  * E rows are stored bf16 (scan state stays fp32 internally), so the
    window-min TensorTensor runs in the DVE 2x_1p mode (2 elem/cycle).
    dist stays fp32.  Validated: L2 rel err ~3e-3 (tolerance 2e-2).
  * dist'^2 comes from a single K=33 fp32r matmul per 512-col chunk
    (fp32r = 1 cycle/row vs fp32's 4): rhs rows = x*-2w2inv (24),
    x2*w2inv (8), w2inv (1); lhsT = block-diag patts / batch indicators /
    p2+eps, one lhsT per group.  EPS=1.5e-2 floors d^2 against fp32r
    cancellation noise (HW sqrt(neg)=NaN).
  * ScalarE applies sqrt PSUM->SBUF; DVE is the bottleneck engine at
    ~105us busy (32 scans of 2049 + 30 bf16 window-mins).
"""

import os
import sys

if "/opt/trn_rl_repo" not in sys.path:
    sys.path.insert(0, "/opt/trn_rl_repo")
# the device path runs through jax's axon PJRT backend; make sure a
# harness-pinned JAX_PLATFORMS doesn't hide it (no-op if jax is already up)
if "jax" not in sys.modules and "axon" not in os.environ.get(
    "JAX_PLATFORMS", "axon"
):
    os.environ["JAX_PLATFORMS"] = "axon," + os.environ["JAX_PLATFORMS"]

import numpy as np

NB, ND, NP, NL, NT = 64, 3, 32, 32, 1024   # batch, xdim, n_patts, l_patts, T
NCORES = 8
BPC = NB // NCORES                     # 8 batches per core
RHO = 0.1
W = RHO ** (1.0 / NL)
SHIFT = 512.0
EPS = 1.5e-2                           # floors d^2 against fp32r noise
INF = 1.0e30
LARGE = 1.0e30                         # scan chain-reset boundary value
K = 33                                 # matmul contraction rows
NT2 = 2 * NT + 1                       # merged scan length (1024+1+1024)

_CACHE = {}


def _tables():
    if "tables" not in _CACHE:
        import ml_dtypes

        t = np.arange(NT, dtype=np.float64)
        w2inv = (W ** (-2.0 * (t - SHIFT))).astype(np.float32)
        wpos = (W ** (t - SHIFT)).astype(np.float32)
        W2INV24 = np.ascontiguousarray(np.tile(w2inv[None, :], (24, 1)) * -2.0)
        W2INVP8 = np.ascontiguousarray(np.tile(w2inv[None, :], (8, 1)))
        W2INV1 = np.ascontiguousarray(w2inv[None, :])
        WPOS2 = np.ascontiguousarray(np.tile(wpos[None, None, :], (128, 2, 1))).astype(ml_dtypes.bfloat16)
        EINF1 = np.full((128, 1), INF, np.float32).astype(ml_dtypes.bfloat16)
        _CACHE["tables"] = (W2INV24, W2INVP8, W2INV1, WPOS2, EINF1)
    return _CACHE["tables"]


def _lhbase(patts):
    """Full stationary lhsT per group: block-diagonal raw patts (rows 0:24,
    the -2 scale lives in the rhs), batch indicators (rows 24:32), and the
    p^2+eps row (row 32, paired with the rhs w2inv row)."""
    lhb = np.zeros((2, K, 128, NL), np.float32)
    pdl = np.transpose(patts, (1, 0, 2))  # (d, p, l)
    p2e = np.einsum("pdl,pdl->pl", patts, patts) + EPS
    for h in range(2):
        for bl in range(4):
            r = 12 * h + 3 * bl
            lhb[h, r : r + 3, 32 * bl : 32 * (bl + 1), :] = pdl
            lhb[h, 24 + 4 * h + bl, 32 * bl : 32 * (bl + 1), :] = 1.0
            lhb[h, 32, 32 * bl : 32 * (bl + 1), :] = p2e
    return lhb


def _build():
    if "nc" in _CACHE:
        return _CACHE["nc"]

    from contextlib import ExitStack

    import concourse.bass as bass  # noqa: F401
    import concourse.tile as tile
    from concourse import bacc, mybir

    f32 = mybir.dt.float32
    f32r = mybir.dt.float32r
    bf16 = mybir.dt.bfloat16
    AOT = mybir.AluOpType

    nc = bacc.Bacc(None, target_bir_lowering=False)
    x8 = nc.dram_tensor("x8", [BPC, ND, NT], f32, kind="ExternalInput")
    lhbase_d = nc.dram_tensor("lhbase", [2, K, 128, NL], f32r, kind="ExternalInput")
    w2inv24_d = nc.dram_tensor("w2inv24", [24, NT], f32, kind="ExternalInput")
    w2invp8_d = nc.dram_tensor("w2invp8", [8, NT], f32, kind="ExternalInput")
    w2inv1_d = nc.dram_tensor("w2inv1", [1, NT], f32r, kind="ExternalInput")
    einf1_d = nc.dram_tensor("einf1", [128, 1], bf16, kind="ExternalInput")
    wpos_d = nc.dram_tensor("wpos2", [128, 2, NT], bf16, kind="ExternalInput")
    out_d = nc.dram_tensor("out8", [BPC, NP, NT], bf16, kind="ExternalOutput")

    with tile.TileContext(nc) as tc:
        with ExitStack() as ctx:
            persist = ctx.enter_context(tc.tile_pool(name="persist", bufs=1))
            c_pool = ctx.enter_context(tc.tile_pool(name="cmin", bufs=2))
            # [128, NT] half-row tiles: sqrt(j,h) only waits on group h's two
            # matmuls, and 4 bufs give two rows of pipeline slack
            psum_pool = ctx.enter_context(
                tc.tile_pool(name="psum", bufs=4, space="PSUM")
            )

            HT = NT // 2
            # lhsT per group: free layout (m, l), l contiguous for patts DMA
            lh = [persist.tile([K, 128, NL], f32r, name=f"lh{h}") for h in range(2)]
            # rhs split by column half so each matmul waits only on its half
            xwh = [persist.tile([K, HT], f32r, name=f"xw{s}") for s in range(2)]
            w2inv24h = [persist.tile([24, HT], f32, name=f"w2inv24{s}") for s in range(2)]
            w2invp8 = persist.tile([8, NT], f32, name="w2invp8")
            xgh = [persist.tile([24, HT], f32, name=f"xg{s}") for s in range(2)]
            xa8 = persist.tile([8, ND, NT], f32, name="xa8")
            x2sum = persist.tile([8, NT], f32, name="x2sum")
            x2w = persist.tile([8, NT], f32r, name="x2w")
            wpos = persist.tile([128, 2, NT], bf16, name="wpos")
            inf2 = persist.tile([128, NT // 2], f32, name="inf2")
            einf = persist.tile([128, 1], bf16, name="einf")
            E0 = persist.tile([128, NT2 + 1], bf16, name="E0")
            E1 = persist.tile([128, NT2 + 1], bf16, name="E1")
            E = [E0, E1]
            d3r = [persist.tile([128, NT2], f32, name=f"d3_{i}") for i in range(4)]
            oth = persist.tile([128, 2, NT], bf16, name="oth")

            # ---------------- startup ----------------
            # critical path to the first scan: xa8 DMA -> squares -> x2sum
            # adds -> x2w -> xw[24:32] DMA -> matmuls -> sqrts.  Everything
            # else (p2e chain, lhbase, tables) is ordered to stay off it.
            actd = persist.tile([1, 1], f32, name="actd")
            nc.vector.memset(actd[:], 1.0)
            nc.scalar.sqrt(actd[:], actd[:])  # preload the Sqrt ACT table

            xgf = x8.rearrange("b d t -> (b d) t")
            # x DMAs split by t-half so the x2 pipeline starts on half 0.
            # HWDGE descriptor gen costs ~625ns per DMA per queue, so the
            # critical-path DMAs are spread: sync carries half-0 + lhbase,
            # scalar carries xa8 + the tiny xw fills, gpsimd (SWDGE, idle)
            # carries everything half-1 and the constants.
            nc.scalar.dma_start(xa8[:, :, 0:HT], x8[:, :, 0:HT])
            nc.scalar.dma_start(xa8[:, :, HT:NT], x8[:, :, HT:NT])
            nc.sync.dma_start(xgh[0][:], xgf[:, 0:HT])
            nc.sync.dma_start(w2inv24h[0][:], w2inv24_d[:, 0:HT])
            nc.sync.dma_start(w2invp8[:], w2invp8_d[:])
            nc.sync.dma_start(xwh[0][32:33, :], w2inv1_d[:, 0:HT])
            nc.sync.dma_start(xwh[1][32:33, :], w2inv1_d[:, HT:NT])
            nc.sync.dma_start(lh[0][:, :, :], lhbase_d[0])
            nc.sync.dma_start(xgh[1][:], xgf[:, HT:NT])
            nc.sync.dma_start(w2inv24h[1][:], w2inv24_d[:, HT:NT])
            nc.sync.dma_start(lh[1][:, :, :], lhbase_d[1])
            nc.gpsimd.dma_start(einf[:], einf1_d[:])

            # rhs (xw) build, pipelined by t-half:
            # square -> two adds -> w2inv mult -> DMA into xw rows 24:32
            for s in range(2):
                ts0, ts1 = s * HT, (s + 1) * HT
                nc.scalar.square(xa8[:, :, ts0:ts1], xa8[:, :, ts0:ts1])
                nc.vector.tensor_tensor(
                    xwh[s][0:24, :], xgh[s][:], w2inv24h[s][:], op=AOT.mult
                )
                nc.vector.tensor_tensor(
                    x2sum[:, ts0:ts1], xa8[:, 0, ts0:ts1], xa8[:, 1, ts0:ts1],
                    op=AOT.add,
                )
                nc.vector.tensor_tensor(
                    x2sum[:, ts0:ts1], x2sum[:, ts0:ts1], xa8[:, 2, ts0:ts1],
                    op=AOT.add,
                )
                nc.vector.tensor_tensor(
                    x2w[:, ts0:ts1], x2sum[:, ts0:ts1], w2invp8[:, ts0:ts1],
                    op=AOT.mult,
                )
                nc.scalar.dma_start(xwh[s][24:32, :], x2w[:, ts0:ts1])

            # ---------------- DP state init (DVE fills its idle window) ----
            nc.vector.memset(inf2[:], INF)
            # E1 edge slots; E0's edges are only read via the j==0 stash
            nc.vector.tensor_copy(out=E1[:, 0:1], in_=einf[:])
            nc.vector.tensor_copy(out=E1[:, NT + 1 : NT + 2], in_=einf[:])


            # ---------------- main loop over DP rows ----------------
            for j in range(NL):
                d3 = d3r[j % 4]
                if j <= 1:
                    # four independent matmul->sqrt quarter pipelines;
                    # column-half 0 (ready first) for both groups, then half 1
                    for q in range(2):
                        for h in range(2):
                            k0 = 0 if h == 0 else NT + 1
                            psq = psum_pool.tile([128, NT], f32, name="ps")
                            nc.tensor.matmul(
                                psq[:, 0:HT], lh[h][:, :, j], xwh[q][:, :],
                                start=True, stop=True,
                            )
                            nc.scalar.sqrt(
                                d3[:, k0 + q * HT : k0 + (q + 1) * HT],
                                psq[:, 0:HT],
                            )
                else:
                    for h in range(2):
                        k0 = 0 if h == 0 else NT + 1
                        ps = psum_pool.tile([128, NT], f32, name="ps")
                        nc.tensor.matmul(
                            ps[:, 0:HT], lh[h][:, :, j], xwh[0][:, :],
                            start=True, stop=True,
                        )
                        nc.tensor.matmul(
                            ps[:, HT:NT], lh[h][:, :, j], xwh[1][:, :],
                            start=True, stop=True,
                        )
                        nc.scalar.sqrt(d3[:, k0 : k0 + NT], ps[:, :])

                Ecur, Eprev = E[j % 2], E[(j + 1) % 2]
                if j == 0:
                    # chunked cumsum chained via `initial`, following the
                    # quarter sqrts down the pipeline
                    for q in range(2):
                        for h in range(2):
                            lo = (1 if h == 0 else NT + 2) + q * HT
                            k0 = (0 if h == 0 else NT + 1) + q * HT
                            nc.vector.tensor_tensor_scan(
                                out=Ecur[:, lo : lo + HT],
                                data0=inf2[:],
                                data1=d3[:, k0 : k0 + HT],
                                initial=0.0 if q == 0 else Ecur[:, lo - 1 : lo],
                                op0=AOT.min,
                                op1=AOT.add,
                            )
                    # stash E0[t=0] per group into the edge slots so row 1's
                    # shifted-data0 view (cumsum monotonicity shortcut) is
                    # exact at t=0
                    nc.vector.tensor_copy(out=Ecur[:, 0:1], in_=Ecur[:, 1:2])
                    nc.vector.tensor_copy(
                        out=Ecur[:, NT + 1 : NT + 2], in_=Ecur[:, NT + 2 : NT + 3]
                    )
                elif j == 1:
                    # min(E0[t], E0[t-1]) == E0[t-1] by monotonicity; quarter
                    # chunks chase the quarter sqrts
                    for q in range(2):
                        for h in range(2):
                            lo = (1 if h == 0 else NT + 2) + q * HT
                            k0 = (0 if h == 0 else NT + 1) + q * HT
                            nc.vector.tensor_tensor_scan(
                                out=Ecur[:, lo : lo + HT],
                                data0=Eprev[:, k0 : k0 + HT],
                                data1=d3[:, k0 : k0 + HT],
                                initial=INF if q == 0 else Ecur[:, lo - 1 : lo],
                                op0=AOT.min,
                                op1=AOT.add,
                            )
                    # restore the INF edges for row 2's buffer reuse
                    nc.vector.tensor_copy(out=Eprev[:, 0:1], in_=einf[:])
                    nc.vector.tensor_copy(
                        out=Eprev[:, NT + 1 : NT + 2], in_=einf[:]
                    )
                elif j < NL - 1:
                    # per-group split with order [wminB, wminA, scanB, scanA]:
                    # every op's producer is >=2 DVE ops back, so the
                    # same-engine semaphore round-trips hide behind execution
                    c3 = c_pool.tile([128, NT2], bf16, name="c3")
                    for h in (1, 0):
                        k0 = 0 if h == 0 else NT + 1
                        nc.vector.tensor_tensor(
                            c3[:, k0 : k0 + NT],
                            Eprev[:, k0 + 1 : k0 + 1 + NT],
                            Eprev[:, k0 : k0 + NT],
                            op=AOT.min,
                        )
                    for h in (1, 0):
                        k0 = 0 if h == 0 else NT + 1
                        lo = k0 + 1
                        nc.vector.tensor_tensor_scan(
                            out=Ecur[:, lo : lo + NT],
                            data0=c3[:, k0 : k0 + NT],
                            data1=d3[:, k0 : k0 + NT],
                            initial=INF,
                            op0=AOT.min,
                            op1=AOT.add,
                        )
                else:
                    # last row in quarter chunks: each chunk's rescale +
                    # store overlaps the remaining chunks' scans
                    of = out_d.rearrange("b p t -> (b p) t")
                    engs = [nc.sync, nc.scalar, nc.gpsimd, nc.sync]
                    c3 = c_pool.tile([128, NT2], bf16, name="c3")
                    for hh in (1, 0):
                        k0 = 0 if hh == 0 else NT + 1
                        nc.vector.tensor_tensor(
                            c3[:, k0 : k0 + NT],
                            Eprev[:, k0 + 1 : k0 + 1 + NT],
                            Eprev[:, k0 : k0 + NT],
                            op=AOT.min,
                        )
                    for h in range(2):
                        rows = slice(h * 128, (h + 1) * 128)
                        for q in range(2):
                            lo = (1 if h == 0 else NT + 2) + q * HT
                            k0 = (0 if h == 0 else NT + 1) + q * HT
                            t0 = q * HT
                            nc.vector.tensor_tensor_scan(
                                out=Ecur[:, lo : lo + HT],
                                data0=c3[:, k0 : k0 + HT],
                                data1=d3[:, k0 : k0 + HT],
                                initial=INF if q == 0 else Ecur[:, lo - 1 : lo],
                                op0=AOT.min,
                                op1=AOT.add,
                            )
                            nc.vector.tensor_tensor(
                                oth[:, h, t0 : t0 + HT],
                                Ecur[:, lo : lo + HT],
                                wpos[:, h, t0 : t0 + HT],
                                op=AOT.mult,
                            )
                            engs[2 * h + q].dma_start(
                                of[rows, t0 : t0 + HT], oth[:, h, t0 : t0 + HT]
                            )
                if j == 0:
                    # 1MB table only needed by the output stage; load it
                    # behind the startup DMAs
                    nc.gpsimd.dma_start(wpos[:], wpos_d[:])

    nc.compile()
    _CACHE["nc"] = nc
    return nc


def _in_maps(x, patts):
    W2INV24, W2INVP8, W2INV1, WPOS2, EINF1 = _tables()
    x = np.ascontiguousarray(np.asarray(x, dtype=np.float32))
    patts = np.ascontiguousarray(np.asarray(patts, dtype=np.float32))
    lhb = _lhbase(patts)
    maps = []
    for c in range(NCORES):
        maps.append(
            {
                "x8": np.ascontiguousarray(x[c * BPC : (c + 1) * BPC]),
                "lhbase": lhb,
                "w2inv24": W2INV24,
                "w2invp8": W2INVP8,
                "w2inv1": W2INV1,
                "einf1": EINF1,
                "wpos2": WPOS2,
            }
        )
    return maps


def kernel(x, patts):
    nc = _build()
    from concourse.bass_utils import run_bass_kernel_spmd

    res = run_bass_kernel_spmd(
        nc, _in_maps(x, patts), core_ids=list(range(NCORES))
    )
    _CACHE["last_results"] = res
    out = np.concatenate([r["out8"] for r in res.results], axis=0)
    return out.astype(np.float32)
